# revision 31
# baseline (speedup 1.0000x reference)
"""Channel-wise row attention kernel for Trainium2 (8 NeuronCores).

Reference computation (per (n, w) slab, with qp = q[n,:,:,w].T etc. of shape (H, C)):
    attn = softmax(qp @ kp.T / sqrt(C), axis=-1);  out_slab = (attn @ vp).T  # (C, H)

Sharding: (n, w-quarter) across 8 cores -> each core owns 64 independent slabs.
Host pre-permutes inputs to a LINEAR device layout (every group DMA is one
fully-contiguous 8KB-per-partition run) and casts to bf16:
    q_dev, k_dev: (C, 64, H) bf16   v_dev: (128p, 64, GT, C) bf16   o: (C, 64, H) bf16

Device (per slab; partition-dim softmax, no on-chip transposes). HW-ablation
findings that shaped this (R-slope measured, 2026-08-10):
  * nc.vector.reciprocal is an ~8 cyc/elem iterative divide: 3.7us/slab, and
    was the silent whole-kernel bottleneck (~285us). reciprocal_approx_fast
    (~51 ULP) is ~5x faster and error-budget-irrelevant.
  * ScalarE ACTIVATE costs (FD+352cyc)/1.2GHz per instruction: exp of the 4
    S-tiles is issued as 2 fused FD=1024 activations over 2-bank PSUM pairs.
  * One DVE bf16 add folds the two E-pairs (ea = e01+e23), cutting the
    colsum matmuls from 4 to 2 (PE is the binding engine at ~256ns/matmul).
  * Software pipelining: the post-exp tail (colsum MMs, recip, O MMs, mul)
    of slab j is emitted during slab j+1's window, so the PE queue never
    head-of-line blocks on the exp -> fold chain (202us -> 181us).
Per-slab steady state: PE 10 matmuls (4 S + 2 colsum + 4 O, all bf16, f32
PSUM), ScalarE 2 exps, DVE fold + approx-recip + normalize-mul, PSUM split
2x(2-bank s-pairs) + 2 cs + 2 o = 8 banks.
"""

import numpy as np
from contextlib import ExitStack

import concourse.bass as bass
import concourse.bacc as bacc
import concourse.tile as tile
import concourse.mybir as mybir
from concourse.bass_utils import run_bass_kernel_spmd

N, C, H, W = 2, 128, 512, 256
NCORES = 8
WQ = 4                 # w-quarters per n
WPC = W // WQ          # 64 slabs per core
GT = H // 128          # 4 g-tiles per slab
SCALE = float(1.0 / np.sqrt(np.float32(C)))
F32 = mybir.dt.float32
BF16 = mybir.dt.bfloat16
import ml_dtypes
BFDT = ml_dtypes.bfloat16


def _body(
    ctx: ExitStack,
    tc: tile.TileContext,
    qd,
    kd,
    vd,
    od,
    n_slabs: int,
    group: int,
    repeat: int = 1,
):
    nc = tc.nc
    import os
    fuse = os.environ.get("KB_FUSE", "pair")
    in_bufs = int(os.environ.get("KB_IN_BUFS", "3"))
    out_bufs = int(os.environ.get("KB_OUT_BUFS", "2"))
    e_bufs = int(os.environ.get("KB_E_BUFS", "2"))
    if fuse == "pair":
        # s tiles are 2 PSUM banks each: 2*2 + 2 + 2 = 8 banks total.
        ps_s_bufs = int(os.environ.get("KB_PS_S", "2"))
        ps_cs_bufs = int(os.environ.get("KB_PS_CS", "2"))
        ps_o_bufs = int(os.environ.get("KB_PS_O", "2"))
    else:
        ps_s_bufs = int(os.environ.get("KB_PS_S", "3"))
        ps_cs_bufs = int(os.environ.get("KB_PS_CS", "2"))
        ps_o_bufs = int(os.environ.get("KB_PS_O", "3"))
    const_pool = ctx.enter_context(tc.tile_pool(name="const", bufs=1))
    in_pool = ctx.enter_context(tc.tile_pool(name="inp", bufs=in_bufs))
    e_pool = ctx.enter_context(tc.tile_pool(name="epool", bufs=e_bufs))
    r_pool = ctx.enter_context(tc.tile_pool(name="rpool", bufs=int(os.environ.get("KB_R_BUFS", "2"))))
    out_pool = ctx.enter_context(tc.tile_pool(name="outp", bufs=out_bufs))
    ps_s = ctx.enter_context(tc.tile_pool(name="ps_s", bufs=ps_s_bufs, space="PSUM"))
    ps_cs = ctx.enter_context(tc.tile_pool(name="ps_cs", bufs=ps_cs_bufs, space="PSUM"))
    ps_o = ctx.enter_context(tc.tile_pool(name="ps_o", bufs=ps_o_bufs, space="PSUM"))

    ones_t = const_pool.tile([128, 128], BF16, name="ones_t")
    nc.vector.memset(ones_t, 1.0)

    lin = os.environ.get("KB_LAYOUT", "lin") == "lin"

    def od_slice(ww0):
        if lin:
            return od[:, ww0 : ww0 + group, :]
        return od[ww0 : ww0 + group].rearrange("s c h -> c s h")

    n_groups = n_slabs // group
    pending_mul = None
    pending_t = None
    for gi in range(n_groups * repeat):
        gi = gi % n_groups
        w0 = gi * group
        _abl = os.environ.get("KB_ABL", "")
        if lin:
            # Linear layout: DRAM tensors are pre-permuted on host so each
            # group transfer is one fully-contiguous 8KB-per-partition run.
            q_g = in_pool.tile([C, group, H], BF16, tag="q", name="q_g")
            k_g = in_pool.tile([C, group, H], BF16, tag="k", name="k_g")
            v_g = in_pool.tile([128, group, GT, C], BF16, tag="v", name="v_g")
            if _abl not in ("nodma", "mmonly"):
                nc.sync.dma_start(out=q_g, in_=qd[:, w0 : w0 + group, :])
                _kdma = (
                    nc.scalar if os.environ.get("KB_SPLIT_DMA", "0") == "1" else nc.sync
                )
                _kdma.dma_start(out=k_g, in_=kd[:, w0 : w0 + group, :])
                nc.sync.dma_start(out=v_g, in_=vd[:, w0 : w0 + group, :, :])
        else:
            # Group DMAs: (c, s, h) tiles; per-(c,s) runs of H are contiguous in DRAM.
            q_g = in_pool.tile([C, group, H], BF16, tag="q", name="q_g")
            nc.sync.dma_start(
                out=q_g, in_=qd[w0 : w0 + group].rearrange("s c h -> c s h")
            )
            k_g = in_pool.tile([C, group, H], BF16, tag="k", name="k_g")
            _kdma = nc.scalar if os.environ.get("KB_SPLIT_DMA", "0") == "1" else nc.sync
            _kdma.dma_start(out=k_g, in_=kd[w0 : w0 + group].rearrange("s c h -> c s h"))
            # v: (s, (t p), c) -> partitions p, free (s, t, c); c-runs (512B) contiguous.
            v_g = in_pool.tile([128, group, GT, C], BF16, tag="v", name="v_g")
            nc.sync.dma_start(
                out=v_g,
                in_=vd[w0 : w0 + group].rearrange("s (t p) c -> p s t c", p=128),
            )
        out_g = out_pool.tile([C, group, H], BF16, tag="out", name="out_g")

        if fuse == "pair":
            # Fused-exp path: S is computed into 2-bank PSUM pair tiles and
            # exp'd with ONE activation per pair (FD=1024). ScalarE pays its
            # ~352-cycle per-instruction bubble twice per slab instead of 4x.
            abl = os.environ.get("KB_ABL", "")
            e_const = s_const = None
            if abl in ("noexp", "dveonly", "mmonly"):
                e_const = const_pool.tile([128, 2, H], BF16, name="e_const")
                nc.vector.memset(e_const, 0.01)
            if abl == "mmonly":
                nc.vector.memset(q_g, 0.25)
                nc.vector.memset(k_g, 0.25)
                nc.vector.memset(v_g, 0.25)
            if abl in ("actonly", "dveonly", "nocs", "reciponly", "recaponly", "mulonly"):
                s_const = const_pool.tile([128, 2, H], F32, name="s_const")
                nc.vector.memset(s_const, 0.5)
            samew = abl == "samew"
            defer = os.environ.get("KB_DEFER", "1") == "1" and not abl
            if abl == "dmaonly":
                nc.vector.memset(out_g, 0.0)

            def emit_tail(pend):
                # cs + recip + O + mul for a previous slab: all inputs (ea,
                # e2 pairs) are ready, so these PE ops never stall the queue.
                p_e2s, p_ea, p_vg, p_outg, p_j, p_w0 = pend
                cs_ps = ps_cs.tile([128, H], F32, tag="cs", name="cs_ps")
                for i in range(2):
                    nc.tensor.matmul(
                        cs_ps,
                        lhsT=ones_t,
                        rhs=p_ea[:, i, :],
                        start=(i == 0),
                        stop=(i == 1),
                    )
                r_t = r_pool.tile([128, H], F32, tag="r", name="r_t")
                if os.environ.get("KB_RECIP2", "approx") == "approx":
                    nc.vector.reciprocal_approx_fast(out=r_t, in_=cs_ps)
                else:
                    nc.vector.reciprocal(r_t, cs_ps)
                o_ps = ps_o.tile([128, H], F32, tag="o", name="o_ps")
                for p in range(2):
                    for i in range(2):
                        t = 2 * p + i
                        nc.tensor.matmul(
                            o_ps,
                            lhsT=p_vg[:, p_j, t, :],
                            rhs=p_e2s[p][:, i, :],
                            start=(p == 0 and i == 0),
                            stop=(p == 1 and i == 1),
                        )
                nc.vector.tensor_mul(p_outg[:, p_j, :], o_ps, r_t)
                if p_j == group - 1:
                    nc.gpsimd.dma_start(out=od_slice(p_w0), in_=p_outg)
            for j in range(group):
                q_t = q_g[:, j, :]
                k_t = k_g[:, j, :]
                e2s = []
                if abl == "dmaonly":
                    continue
                if abl == "dveonly":
                    r_t = r_pool.tile([128, H], F32, tag="r", name="r_t")
                    nc.vector.reciprocal(r_t, s_const[:, 0, :])
                    nc.vector.tensor_mul(out_g[:, j, :], s_const[:, 1, :], r_t)
                    continue
                if abl in ("reciponly", "recaponly", "mulonly"):
                    r_t = r_pool.tile([128, H], F32, tag="r", name="r_t")
                    if abl == "reciponly":
                        nc.vector.reciprocal(r_t, s_const[:, 0, :])
                    elif abl == "recaponly":
                        nc.vector.reciprocal_approx_fast(out=r_t, in_=s_const[:, 0, :])
                    else:
                        nc.vector.tensor_mul(out_g[:, j, :], s_const[:, 1, :], s_const[:, 0, :])
                    continue
                if abl == "actonly":
                    for p in range(2):
                        e2 = e_pool.tile([128, 2, H], BF16, tag=f"e{p}", name="e2")
                        nc.scalar.activation(
                            e2, s_const, mybir.ActivationFunctionType.Exp, scale=SCALE
                        )
                    continue
                for p in range(2):
                    s2 = ps_s.tile([128, 2, H], F32, tag="s", name="s2")
                    for i in range(2):
                        t = 2 * p + i
                        nc.tensor.matmul(
                            s2[:, i, :],
                            lhsT=ones_t if samew else k_t[:, t * 128 : (t + 1) * 128],
                            rhs=q_t,
                            start=True,
                            stop=True,
                        )
                    if abl in ("noexp", "sonly", "mmonly"):
                        e2s.append(e_const)
                        continue
                    e2 = e_pool.tile([128, 2, H], BF16, tag=f"e{p}", name="e2")
                    nc.scalar.activation(
                        e2, s2, mybir.ActivationFunctionType.Exp, scale=SCALE
                    )
                    e2s.append(e2)
                if abl == "sonly":
                    continue
                if defer:
                    # fold emitted before the previous slab's tail: HW-measured
                    # faster than tail-first (181.6us vs 189.8us).
                    ea = e_pool.tile([128, 2, H], BF16, tag="ea", name="ea")
                    if os.environ.get("KB_FOLD_ENG", "vector") == "gpsimd":
                        nc.gpsimd.scalar_tensor_tensor(
                            out=ea,
                            in0=e2s[0],
                            scalar=1.0,
                            in1=e2s[1],
                            op0=mybir.AluOpType.mult,
                            op1=mybir.AluOpType.add,
                        )
                    else:
                        nc.vector.tensor_add(ea, e2s[0], e2s[1])
                    if pending_t is not None:
                        emit_tail(pending_t)
                    pending_t = (e2s, ea, v_g, out_g, j, w0)
                    continue
                cs_ps = ps_cs.tile([128, H], F32, tag="cs", name="cs_ps")
                if abl == "nocs":
                    r_t = r_pool.tile([128, H], F32, tag="r", name="r_t")
                    nc.vector.reciprocal(r_t, s_const[:, 0, :])
                elif os.environ.get("KB_CS_FOLD2", "0") == "1":
                    # One DVE add folds the two pairs (bf16 2x mode, FD=1024);
                    # colsum needs only 2 matmuls on the folded pair.
                    ea = e_pool.tile([128, 2, H], BF16, tag="ea", name="ea")
                    nc.vector.tensor_add(ea, e2s[0], e2s[1])
                    for i in range(2):
                        nc.tensor.matmul(
                            cs_ps,
                            lhsT=ones_t,
                            rhs=ea[:, i, :],
                            start=(i == 0),
                            stop=(i == 1),
                        )
                else:
                    for p in range(2):
                        for i in range(2):
                            nc.tensor.matmul(
                                cs_ps,
                                lhsT=ones_t,
                                rhs=e2s[p][:, i, :],
                                start=(p == 0 and i == 0),
                                stop=(p == 1 and i == 1),
                            )
                if abl not in ("nocs", "mmonly"):
                    r_t = r_pool.tile([128, H], F32, tag="r", name="r_t")
                    if os.environ.get("KB_RECIP2", "approx") == "approx":
                        # DVE exact reciprocal is an 8-cycle/elem iterative
                        # divide (~4.3us/slab, the whole-kernel bottleneck).
                        # The NR-seeded approx (~51 ULP) is ~5x faster; the
                        # 2e-2 error budget doesn't notice 2e-5 rel error.
                        nc.vector.reciprocal_approx_fast(out=r_t, in_=cs_ps)
                    else:
                        nc.vector.reciprocal(r_t, cs_ps)
                o_ps = ps_o.tile([128, H], F32, tag="o", name="o_ps")
                for p in range(2):
                    for i in range(2):
                        t = 2 * p + i
                        nc.tensor.matmul(
                            o_ps,
                            lhsT=ones_t if samew else v_g[:, j, t, :],
                            rhs=e2s[p][:, i, :],
                            start=(p == 0 and i == 0),
                            stop=(p == 1 and i == 1),
                        )
                if abl != "mmonly":
                    nc.vector.tensor_mul(out_g[:, j, :], o_ps, r_t)
            if not defer and abl not in (
                "sonly", "actonly", "nodma", "noout", "reciponly", "recaponly", "mmonly",
            ):
                nc.gpsimd.dma_start(out=od_slice(w0), in_=out_g)
            continue
        for j in range(group):
            q_t = q_g[:, j, :]
            k_t = k_g[:, j, :]
            e_ts = []
            for t in range(GT):
                s_ps = ps_s.tile([128, H], F32, tag="s", name="s_ps")
                nc.tensor.matmul(
                    s_ps,
                    lhsT=k_t[:, t * 128 : (t + 1) * 128],
                    rhs=q_t,
                    start=True,
                    stop=True,
                )
                e_t = e_pool.tile([128, H], BF16, tag=f"e{t}", name="e_t")
                nc.scalar.activation(
                    e_t, s_ps, mybir.ActivationFunctionType.Exp, scale=SCALE
                )
                e_ts.append(e_t)

            if os.environ.get("KB_DMA_FOLD", "0") == "1":
                # O-matmuls first: they read the original e tiles, after which
                # the fold may clobber e0/e2 in place.
                o_ps = ps_o.tile([128, H], F32, tag="o", name="o_ps")
                for t in range(GT):
                    nc.tensor.matmul(
                        o_ps,
                        lhsT=v_g[:, j, t, :],
                        rhs=e_ts[t],
                        start=(t == 0),
                        stop=(t == GT - 1),
                    )
                # Pairwise fold on the (idle) DMA engines: e0 += e1, e2 += e3.
                nc.gpsimd.dma_start(
                    out=e_ts[0], in_=e_ts[1], accum_op=mybir.AluOpType.add
                )
                nc.gpsimd.dma_start(
                    out=e_ts[2], in_=e_ts[3], accum_op=mybir.AluOpType.add
                )
                cs_ps = ps_cs.tile([128, H], F32, tag="cs", name="cs_ps")
                nc.tensor.matmul(cs_ps, lhsT=ones_t, rhs=e_ts[0], start=True, stop=False)
                nc.tensor.matmul(cs_ps, lhsT=ones_t, rhs=e_ts[2], start=False, stop=True)
                r_t = r_pool.tile([128, H], F32, tag="r", name="r_t")
                nc.vector.reciprocal(r_t, cs_ps)
                nc.vector.tensor_mul(out_g[:, j, :], o_ps, r_t)
                continue
            if os.environ.get("KB_O_BEFORE_CS", "0") == "1":
                o_ps = ps_o.tile([128, H], F32, tag="o", name="o_ps")
                for t in range(GT):
                    nc.tensor.matmul(
                        o_ps,
                        lhsT=v_g[:, j, t, :],
                        rhs=e_ts[t],
                        start=(t == 0),
                        stop=(t == GT - 1),
                    )
                cs_ps = ps_cs.tile([128, H], F32, tag="cs", name="cs_ps")
                for t in range(GT):
                    nc.tensor.matmul(
                        cs_ps,
                        lhsT=ones_t,
                        rhs=e_ts[t],
                        start=(t == 0),
                        stop=(t == GT - 1),
                    )
                r_t = r_pool.tile([128, H], F32, tag="r", name="r_t")
                nc.vector.reciprocal(r_t, cs_ps)
                nc.vector.tensor_mul(out_g[:, j, :], o_ps, r_t)
                continue
            if os.environ.get("KB_DEFER_MUL", "0") == "1":
                cs_ps = ps_cs.tile([128, H], F32, tag="cs", name="cs_ps")
                for t in range(GT):
                    nc.tensor.matmul(
                        cs_ps,
                        lhsT=ones_t,
                        rhs=e_ts[t],
                        start=(t == 0),
                        stop=(t == GT - 1),
                    )
                r_t = r_pool.tile([128, H], F32, tag="r", name="r_t")
                nc.vector.reciprocal(r_t, cs_ps)
                # emit the PREVIOUS slab's normalize-mul after this slab's
                # reciprocal so recip always leads the DVE queue (keeps the
                # cs WAR edge from cascading).
                if pending_mul is not None:
                    p_out, p_j, p_o, p_r, p_gi = pending_mul
                    nc.vector.tensor_mul(p_out[:, p_j, :], p_o, p_r)
                    if p_j == group - 1:
                        pw0 = p_gi * group
                        nc.gpsimd.dma_start(out=od_slice(pw0), in_=p_out)
                o_ps = ps_o.tile([128, H], F32, tag="o", name="o_ps")
                for t in range(GT):
                    nc.tensor.matmul(
                        o_ps,
                        lhsT=v_g[:, j, t, :],
                        rhs=e_ts[t],
                        start=(t == 0),
                        stop=(t == GT - 1),
                    )
                pending_mul = (out_g, j, o_ps, r_t, gi)
                continue
            cs_ps = ps_cs.tile([128, H], F32, tag="cs", name="cs_ps")
            if os.environ.get("KB_CS_FOLD", "0") == "1":
                ea = e_pool.tile([128, H], BF16, tag="ea", name="ea")
                nc.vector.tensor_add(ea, e_ts[0], e_ts[1])
                eb = e_pool.tile([128, H], BF16, tag="eb", name="eb")
                nc.vector.tensor_add(eb, e_ts[2], e_ts[3])
                ec = e_pool.tile([128, H], BF16, tag="ec", name="ec")
                nc.vector.tensor_add(ec, ea, eb)
                nc.tensor.matmul(cs_ps, lhsT=ones_t, rhs=ec, start=True, stop=True)
            else:
                for t in range(GT):
                    nc.tensor.matmul(
                        cs_ps,
                        lhsT=ones_t,
                        rhs=e_ts[t],
                        start=(t == 0),
                        stop=(t == GT - 1),
                    )
            r_t = r_pool.tile([128, H], F32, tag="r", name="r_t")
            if os.environ.get("KB_RECIP", "vector") == "act":
                ln_t = r_pool.tile([128, H], F32, tag="ln", name="ln_t")
                nc.scalar.activation(
                    ln_t, cs_ps, mybir.ActivationFunctionType.Ln
                )
                nc.scalar.activation(
                    r_t, ln_t, mybir.ActivationFunctionType.Exp, scale=-1.0
                )
            else:
                nc.vector.reciprocal(r_t, cs_ps)

            o_ps = ps_o.tile([128, H], F32, tag="o", name="o_ps")
            for t in range(GT):
                nc.tensor.matmul(
                    o_ps,
                    lhsT=v_g[:, j, t, :],
                    rhs=e_ts[t],
                    start=(t == 0),
                    stop=(t == GT - 1),
                )
            nc.vector.tensor_mul(out_g[:, j, :], o_ps, r_t)

        if os.environ.get("KB_DEFER_MUL", "0") != "1":
            nc.gpsimd.dma_start(out=od_slice(w0), in_=out_g)

    if pending_t is not None:
        emit_tail(pending_t)
    if pending_mul is not None:
        p_out, p_j, p_o, p_r, p_gi = pending_mul
        nc.vector.tensor_mul(p_out[:, p_j, :], p_o, p_r)
        pw0 = p_gi * group
        nc.gpsimd.dma_start(out=od_slice(pw0), in_=p_out)


def build_nc(
    n_slabs: int = WPC, group: int = 8, repeat: int = 1, timing_mode: bool = False
) -> bass.Bass:
    """timing_mode: q/k/v become Internal DRAM scratch (contents irrelevant for
    timing; engine timing is data-independent) so the only external input is a
    small seed tensor -- removes host->device transfer from wall-clock."""
    nc = bacc.Bacc("TRN2", target_bir_lowering=False, debug=False)
    import os
    group = int(os.environ.get("KB_GROUP", str(group)))
    lin = os.environ.get("KB_LAYOUT", "lin") == "lin"
    kind = "Internal" if timing_mode else "ExternalInput"
    if lin:
        q_shape = k_shape = [C, n_slabs, H]
        v_shape = [128, n_slabs, GT, C]
        o_shape = [C, n_slabs, H]
    else:
        q_shape = k_shape = [n_slabs, C, H]
        v_shape = [n_slabs, H, C]
        o_shape = [n_slabs, C, H]
    qd = nc.dram_tensor("qi" if timing_mode else "q", q_shape, BF16, kind=kind).ap()
    kd = nc.dram_tensor("ki" if timing_mode else "k", k_shape, BF16, kind=kind).ap()
    vd = nc.dram_tensor("vi" if timing_mode else "v", v_shape, BF16, kind=kind).ap()
    seed = osmall = None
    if timing_mode:
        od = nc.dram_tensor("oi", o_shape, BF16, kind="Internal").ap()
        seed = nc.dram_tensor("seed", [128, 128], BF16, kind="ExternalInput").ap()
        osmall = nc.dram_tensor("osmall", [128, 128], F32, kind="ExternalOutput").ap()
    else:
        od = nc.dram_tensor("o", o_shape, BF16, kind="ExternalOutput").ap()
    with tile.TileContext(nc) as tc, ExitStack() as ctx:
        if timing_mode:
            # Fill internal q/k/v fully with real (small) values: garbage fp32
            # would generate NaN/Inf runtime notifications that distort timing.
            sp = ctx.enter_context(tc.tile_pool(name="seedp", bufs=1))
            st = sp.tile([128, 128], BF16, name="st")
            nc.sync.dma_start(out=st, in_=seed)
            st_b = bass.AP(
                tensor=st.tensor,
                offset=st.offset,
                ap=[list(st.ap[0]), [0, GT], list(st.ap[-1])],
            )
            for s in range(n_slabs):
                if lin:
                    nc.gpsimd.dma_start(
                        out=qd[:, s, :].rearrange("c (t f) -> c t f", f=128), in_=st_b
                    )
                    nc.gpsimd.dma_start(
                        out=kd[:, s, :].rearrange("c (t f) -> c t f", f=128), in_=st_b
                    )
                    nc.gpsimd.dma_start(out=vd[:, s, :, :], in_=st_b)
                else:
                    nc.gpsimd.dma_start(
                        out=qd[s].rearrange("c (t f) -> c t f", f=128), in_=st_b
                    )
                    nc.gpsimd.dma_start(
                        out=kd[s].rearrange("c (t f) -> c t f", f=128), in_=st_b
                    )
                    nc.gpsimd.dma_start(
                        out=vd[s].rearrange("(t p) c -> p t c", p=128), in_=st_b
                    )
            if repeat > 1:
                with tc.For_i(0, repeat, 1):
                    _body(ctx, tc, qd, kd, vd, od, n_slabs, group, 1)
            else:
                _body(ctx, tc, qd, kd, vd, od, n_slabs, group, 1)
            st2 = sp.tile([128, 128], F32, name="st2")
            nc.vector.memset(st2, 2.0)
            nc.sync.dma_start(out=osmall, in_=st2)
        else:
            _body(ctx, tc, qd, kd, vd, od, n_slabs, group, repeat)
    nc.compile()
    return nc


def shard_inputs(q: np.ndarray, k: np.ndarray, v: np.ndarray) -> list[dict]:
    """Host-side shard + permute: core i gets n = i // WQ, w in [64*(i%WQ), ...)."""
    import os

    lin = os.environ.get("KB_LAYOUT", "lin") == "lin"
    in_maps = []
    for i in range(NCORES):
        n, wq = divmod(i, WQ)
        ws = slice(wq * WPC, (wq + 1) * WPC)
        if lin:
            # (C, H, W') -> (C, W', H)
            qs = np.ascontiguousarray(
                np.transpose(q[n, :, :, ws], (0, 2, 1)).astype(BFDT)
            )
            ks = np.ascontiguousarray(
                np.transpose(k[n, :, :, ws], (0, 2, 1)).astype(BFDT)
            )
            # (C, H, W') -> (p, W', GT, C) with H = (GT, p)
            vs = np.ascontiguousarray(
                np.transpose(
                    v[n, :, :, ws].reshape(C, GT, 128, WPC), (2, 3, 1, 0)
                ).astype(BFDT)
            )
        else:
            # (C, H, W') -> (W', C, H)
            qs = np.ascontiguousarray(
                np.transpose(q[n, :, :, ws], (2, 0, 1)).astype(BFDT)
            )
            ks = np.ascontiguousarray(
                np.transpose(k[n, :, :, ws], (2, 0, 1)).astype(BFDT)
            )
            # v pre-transposed: (W', H, C)
            vs = np.ascontiguousarray(
                np.transpose(v[n, :, :, ws], (2, 1, 0)).astype(BFDT)
            )
        in_maps.append({"q": qs, "k": ks, "v": vs})
    return in_maps


def unshard_output(results: list[dict]) -> np.ndarray:
    import os

    lin = os.environ.get("KB_LAYOUT", "lin") == "lin"
    out = np.empty((N, C, H, W), dtype=np.float32)
    for i in range(NCORES):
        n, wq = divmod(i, WQ)
        ws = slice(wq * WPC, (wq + 1) * WPC)
        if lin:
            # o: (C, W', H) -> (C, H, W')
            out[n, :, :, ws] = np.transpose(
                results[i]["o"].astype(np.float32), (0, 2, 1)
            )
        else:
            out[n, :, :, ws] = np.transpose(
                results[i]["o"].astype(np.float32), (1, 2, 0)
            )
    return out


_NC_CACHE = {}


def kernel(q: np.ndarray, k: np.ndarray, v: np.ndarray, **run_kwargs) -> np.ndarray:
    q = np.asarray(q, dtype=np.float32)
    k = np.asarray(k, dtype=np.float32)
    v = np.asarray(v, dtype=np.float32)
    key = "default"
    if key not in _NC_CACHE:
        _NC_CACHE[key] = build_nc()
    nc = _NC_CACHE[key]
    in_maps = shard_inputs(q, k, v)
    res = run_bass_kernel_spmd(nc, in_maps, core_ids=list(range(NCORES)), **run_kwargs)
    out = unshard_output(res.results)
    if run_kwargs.get("trace"):
        kernel.last_result = res
    return out

